# revision 3
# baseline (speedup 1.0000x reference)
"""Sliding-window GQA attention (B=1, S=4096, HID=1024, H=16, KV=4, D=64, W=512)
sharded across 8 trn2 NeuronCores by sequence (512 query rows/core + 512-row
k/v halo recomputed locally).

v2: bf16 compute (DVE 2x, half the HBM bytes), merged DMAs (~34 vs 167),
window mask as post-exp 0/1 multiplies (DVE/GpSimd) instead of PE mask-add
matmuls, sink folded into the denominator (no per-block sink DMAs), rope
pair-sum folded before the PE transposes (half the transpose matmuls),
denominator broadcast via PE then one wide reciprocal, software-pipelined
attention blocks (scores run ahead of AV), weights hoisted+preloaded, and
A1/A2 sharing PSUM pools to avoid phase-boundary WAR stalls.

Self-contained: takes full inputs, shards on host, runs one SPMD Bass kernel
on cores 0-7, reassembles the full output.
"""
import sys
sys.path.insert(0, '/opt/trn_rl_repo')
import numpy as np
import ml_dtypes

import concourse.bass as bass
import concourse.bacc as bacc
import concourse.mybir as mybir
import concourse.hw_specs as _hw_specs

# Route every Ln/Exp activation to the single set that contains both
# ("natural_log_exp_and_others"), so the scheduler's interleaving of Ln and
# Exp ops never forces an ACT table reload (~2.7us each).
_orig_get_act_tables = _hw_specs.get_activation_tables


def _merged_act_tables(arch):
    t = dict(_orig_get_act_tables(arch))
    strip = {mybir.ActivationFunctionType.Ln, mybir.ActivationFunctionType.Exp,
             mybir.ActivationFunctionType.Square}
    for name, fns in t.items():
        if name != "natural_log_exp_and_others":
            t[name] = fns - strip
    return t


bacc.get_activation_tables = _merged_act_tables
import concourse.tile as tile
from concourse.bass_utils import run_bass_kernel_spmd

F32 = mybir.dt.float32
F32R = mybir.dt.float32r
BF16 = mybir.dt.bfloat16
AF = mybir.ActivationFunctionType
OP = mybir.AluOpType
BF = ml_dtypes.bfloat16

B, S, HID = 1, 4096, 1024
H, KV, D = 16, 4, 64
WINDOW = 512
EPS = 1e-5
SCALE = 0.125  # 1/sqrt(D)
NCORE = 8
SLOC = 512    # query rows per core
SKV = 1024    # k/v rows per core (halo + own)

_cache = {}


def _build(phases="ABC"):
    nc = bacc.Bacc("TRN2", target_bir_lowering=False, debug=False, num_devices=NCORE)

    xTd = nc.dram_tensor("xT", [128, 8 * SKV], F32R, kind="ExternalInput").ap()
    wqd = nc.dram_tensor("wq", [128, 8192], F32R, kind="ExternalInput").ap()
    wkvd = nc.dram_tensor("wkv", [128, 4096], F32R, kind="ExternalInput").ap()
    wod = nc.dram_tensor("wo", [128, 8192], F32R, kind="ExternalInput").ap()
    cqsqd = nc.dram_tensor("cqsq", [128, 512], BF16, kind="ExternalInput").ap()
    ckskd = nc.dram_tensor("cksk", [128, 1024], BF16, kind="ExternalInput").ap()
    maskd = nc.dram_tensor("mask01", [128, 1024], BF16, kind="ExternalInput").ap()
    identd = nc.dram_tensor("ident", [128, 128], F32R, kind="ExternalInput").ap()
    sinkrd = nc.dram_tensor("sinkr", [1, 8192], F32, kind="ExternalInput").ap()
    vcold = nc.dram_tensor("vcol", [128, 8], BF16, kind="ExternalInput").ap()
    outd = nc.dram_tensor("out", [SLOC, HID], F32, kind="ExternalOutput").ap()

    with tile.TileContext(nc) as tc:
        with tc.tile_pool(name="const", bufs=1) as cpool, \
             tc.tile_pool(name="persist", bufs=1) as pers:

            # ---------------- constants ----------------
            identb = cpool.tile([128, 128], F32R, tag="ident")
            nc.sync.dma_start(identb[:], identd)
            onesrow = cpool.tile([1, 128], F32R, tag="onesrow")
            nc.vector.memset(onesrow[:].bitcast(F32), 1.0)
            epsc = cpool.tile([128, 1], F32, tag="epsc")
            nc.vector.memset(epsc[:], EPS)
            mask01 = cpool.tile([128, 1024], BF16, tag="mask01")
            nc.sync.dma_start(mask01[:], maskd)
            sinkr = cpool.tile([1, 8192], F32, tag="sinkr")
            nc.sync.dma_start(sinkr[:], sinkrd)
            vcol = cpool.tile([128, 8], BF16, tag="vcol")
            nc.sync.dma_start(vcol[:], vcold)

            # persistent products of phase A
            # v strips: cols per kv head = [ones, d0..d63] (65 wide)
            v_sb = [pers.tile([128, 260], F32R, tag=f"v{st}", name=f"v{st}")
                    for st in range(8)]
            qT_sb = [pers.tile([128, 512], F32R, tag=f"qT{p}", name=f"qT{p}")
                     for p in range(8)]
            # kT: one tile, cols = kv*1024 + t; rows 64:128 duplicate 0:64
            kTd = pers.tile([128, 4096], F32R, tag="kTd", name="kTd")
            aoT_sb = [pers.tile([128, 512], F32R, tag=f"aoT{p}", name=f"aoT{p}")
                      for p in range(8)]

            with tc.tile_pool(name="xpool", bufs=1) as xpool:
                xT_sb = xpool.tile([128, 8 * SKV], F32R, tag="xT", name="xT")
                xv_d = xTd.rearrange("p (k t) -> p k t", k=8)
                xv_s = xT_sb[:].rearrange("p (k t) -> p k t", k=8)

                # ======== phase A1: q projection + norm + rope + transpose ====
                with tc.tile_pool(name="wqp", bufs=1) as wqp, \
                     tc.tile_pool(name="tAq", bufs=3) as tA, \
                     tc.tile_pool(name="psmmq", bufs=4, space="PSUM") as psmm, \
                     tc.tile_pool(name="pstq", bufs=1, space="PSUM") as pst:
                    wq_sb = wqp.tile([128, 8192], F32R, tag="wq", name="wq")
                    wv_d = wqd.rearrange("p (k c) -> p k c", k=8)
                    wv_s = wq_sb[:].rearrange("p (k c) -> p k c", k=8)
                    # load order: wq hf=0 half, own x, wq hf=1 half, halo x
                    nc.sync.dma_start(wv_s[:, :, 0:512], wv_d[:, :, 0:512])
                    nc.sync.dma_start(xv_s[:, :, 512:1024], xv_d[:, :, 512:1024])
                    nc.sync.dma_start(wv_s[:, :, 512:1024], wv_d[:, :, 512:1024])
                    nc.sync.dma_start(xv_s[:, :, 0:512], xv_d[:, :, 0:512])
                    cqsq = wqp.tile([128, 512], BF16, tag="cqsq", name="cqsq")
                    nc.sync.dma_start(cqsq[:], cqsqd)

                    def q_stage1(st, hf):
                        qp = psmm.tile([128, 512], F32, tag="qp", name=f"qp{st}{hf}")
                        for kb in range(8):
                            nc.tensor.matmul(
                                qp[:],
                                xT_sb[:, kb * 1024 + st * 128:kb * 1024 + (st + 1) * 128],
                                wq_sb[:, kb * 1024 + hf * 512:kb * 1024 + (hf + 1) * 512],
                                start=(kb == 0), stop=(kb == 7))
                        sqq = tA.tile([128, 512], F32, tag="sqq", name=f"sqq{st}{hf}")
                        nc.scalar.activation(sqq[:], qp[:], AF.Square)
                        msq = tA.tile([128, 8], F32, tag="msq", name=f"msq{st}{hf}")
                        nc.vector.tensor_reduce(
                            msq[:], sqq[:].rearrange("p (h d) -> p h d", h=8),
                            axis=mybir.AxisListType.X, op=OP.add)
                        return qp, msq

                    def q_stage2(st, hf, qp, msq, qTps):
                        lnq = tA.tile([128, 8], F32, tag="lnq", name=f"lnq{st}{hf}")
                        nc.scalar.activation(lnq[:], msq[:], AF.Ln, bias=epsc[:],
                                             scale=1.0 / D)
                        invq = tA.tile([128, 8], F32, tag="invq", name=f"invq{st}{hf}")
                        nc.scalar.activation(invq[:], lnq[:], AF.Exp, scale=-0.5)
                        qn = tA.tile([128, 512], F32R, tag="qn", name=f"qn{st}{hf}")
                        qnv = qn[:].rearrange("p (h d) -> p h d", h=8)
                        nc.vector.tensor_mul(
                            qnv, qp[:].rearrange("p (h d) -> p h d", h=8),
                            invq[:].unsqueeze(2).broadcast_to([128, 8, D]))
                        j = st - 4
                        ct = cqsq[:, j * 128:j * 128 + 64]
                        stt = cqsq[:, j * 128 + 64:j * 128 + 128]
                        q1 = tA.tile([128, 512], F32R, tag="q1", name=f"q1_{st}{hf}")
                        q1v = q1[:].rearrange("p (h d) -> p h d", h=8)
                        nc.vector.tensor_mul(
                            q1v, qnv, ct.unsqueeze(1).broadcast_to([128, 8, D]))
                        q2 = tA.tile([128, 512], F32R, tag="q2", name=f"q2_{st}{hf}")
                        nc.vector.tensor_mul(
                            q2[:].rearrange("p (h a j) -> p h a j", h=8, a=2),
                            qnv.rearrange("p h (a j) -> p h a j", a=2)[:, :, ::-1, :],
                            stt.rearrange("p (a j) -> p a j", a=2)
                            .unsqueeze(1).broadcast_to([128, 8, 2, 32]))
                        q12 = tA.tile([128, 512], F32R, tag="q12", name=f"q12_{st}{hf}")
                        nc.vector.tensor_add(q12[:], q1[:], q2[:])
                        for db4 in range(4):
                            nc.tensor.matmul(qTps[db4][:, j * 128:(j + 1) * 128],
                                             q12[:, db4 * 128:(db4 + 1) * 128],
                                             identb[:], is_transpose=True)

                    for hf in range(2):
                        qTps = [pst.tile([128, 512], F32R, tag=f"qTps{d}",
                                         name=f"qTps{hf}_{d}") for d in range(4)]
                        pend = None
                        for st in range(4, 8):
                            cur = (st, hf, *q_stage1(st, hf))
                            if pend is not None:
                                q_stage2(*pend, qTps)
                            pend = cur
                        q_stage2(*pend, qTps)
                        for db4 in range(4):
                            nc.vector.tensor_copy(qT_sb[hf * 4 + db4][:], qTps[db4][:])

                # ======== phase A2: k/v projection + norm + rope + transpose ====
                with tc.tile_pool(name="wkp", bufs=1) as wkp, \
                     tc.tile_pool(name="tAk", bufs=3) as tA, \
                     tc.tile_pool(name="psmmk", bufs=4, space="PSUM") as psmm, \
                     tc.tile_pool(name="pstk", bufs=1, space="PSUM") as pst:
                    wkv_sb = wkp.tile([128, 4096], F32R, tag="wkv", name="wkv")
                    nc.sync.dma_start(wkv_sb[:], wkvd)
                    cksk = wkp.tile([128, 1024], BF16, tag="cksk", name="cksk")
                    nc.sync.dma_start(cksk[:], ckskd)

                    def k_stage1(st):
                        kvp = psmm.tile([128, 512], F32, tag="qp", name=f"kvp{st}")
                        for kb in range(8):
                            nc.tensor.matmul(
                                kvp[:],
                                xT_sb[:, kb * 1024 + st * 128:kb * 1024 + (st + 1) * 128],
                                wkv_sb[:, kb * 512:(kb + 1) * 512],
                                start=(kb == 0), stop=(kb == 7))
                        # v: cols kv*65 gets ones, kv*65+1..65 gets v dims
                        nc.scalar.copy(
                            v_sb[st][:].rearrange("p (h d) -> p h d", d=65)[:, :, 1:65],
                            kvp[:, 256:512].rearrange("p (h d) -> p h d", d=64))
                        nc.vector.tensor_copy(
                            v_sb[st][:].rearrange("p (h d) -> p h d", d=65)[:, :, 0:1],
                            vcol[:, st:st + 1].unsqueeze(1).broadcast_to([128, KV, 1]))
                        sqk = tA.tile([128, 256], F32, tag="sqk", name=f"sqk{st}")
                        nc.scalar.activation(sqk[:], kvp[:, 0:256], AF.Square)
                        msk = tA.tile([128, KV], F32, tag="msk", name=f"msk{st}")
                        nc.vector.tensor_reduce(
                            msk[:], sqk[:].rearrange("p (h d) -> p h d", h=KV),
                            axis=mybir.AxisListType.X, op=OP.add)
                        return kvp, msk

                    def k_stage2(st, kvp, msk, kTps):
                        lnk = tA.tile([128, KV], F32, tag="lnk", name=f"lnk{st}")
                        nc.scalar.activation(lnk[:], msk[:], AF.Ln, bias=epsc[:],
                                             scale=1.0 / D)
                        invk = tA.tile([128, KV], F32, tag="invk", name=f"invk{st}")
                        nc.scalar.activation(invk[:], lnk[:], AF.Exp, scale=-0.5)
                        kn = tA.tile([128, 256], F32R, tag="kn", name=f"kn{st}")
                        knv = kn[:].rearrange("p (h d) -> p h d", h=KV)
                        nc.vector.tensor_mul(
                            knv, kvp[:, 0:256].rearrange("p (h d) -> p h d", h=KV),
                            invk[:].unsqueeze(2).broadcast_to([128, KV, D]))
                        ct = cksk[:, st * 128:st * 128 + 64]
                        stt = cksk[:, st * 128 + 64:st * 128 + 128]
                        k1 = tA.tile([128, 256], F32R, tag="k1", name=f"k1_{st}")
                        nc.vector.tensor_mul(
                            k1[:].rearrange("p (h d) -> p h d", h=KV),
                            knv, ct.unsqueeze(1).broadcast_to([128, KV, D]))
                        k2 = tA.tile([128, 256], F32R, tag="k2", name=f"k2_{st}")
                        nc.vector.tensor_mul(
                            k2[:].rearrange("p (h a j) -> p h a j", h=KV, a=2),
                            knv.rearrange("p h (a j) -> p h a j", a=2)[:, :, ::-1, :],
                            stt.rearrange("p (a j) -> p a j", a=2)
                            .unsqueeze(1).broadcast_to([128, KV, 2, 32]))
                        k12 = tA.tile([128, 256], F32R, tag="k12", name=f"k12_{st}")
                        nc.vector.tensor_add(k12[:], k1[:], k2[:])
                        j = st % 4
                        for kv in range(4):
                            nc.tensor.matmul(kTps[kv][:, j * 128:(j + 1) * 128],
                                             k12[:, kv * 64:(kv + 1) * 64],
                                             identb[:], is_transpose=True)

                    for sh in range(2):
                        kTps = [pst.tile([64, 512], F32R, tag=f"kTps{kv}",
                                         name=f"kTps{sh}_{kv}") for kv in range(4)]
                        pendk = None
                        for st4 in range(4):
                            st = sh * 4 + st4
                            curk = (st, *k_stage1(st))
                            if pendk is not None:
                                k_stage2(*pendk, kTps)
                            pendk = curk
                        k_stage2(*pendk, kTps)
                        for kv in range(4):
                            nc.vector.tensor_copy(
                                kTd[0:64, kv * 1024 + sh * 512:kv * 1024 + (sh + 1) * 512],
                                kTps[kv][:])
                        kv_lo = kTd[0:64, :].rearrange("p (k t) -> p k t", k=4)
                        kv_hi = kTd[64:128, :].rearrange("p (k t) -> p k t", k=4)
                        nc.sync.dma_start(kv_hi[:, :, sh * 512:(sh + 1) * 512],
                                          kv_lo[:, :, sh * 512:(sh + 1) * 512])

            # ======== phase B: attention;  phase C: out-projection ========
            with tc.tile_pool(name="wB", bufs=1) as wB, \
                 tc.tile_pool(name="sbB", bufs=2) as sbB:
                wo_sb = wB.tile([128, 8192], F32R, tag="wo", name="wo")
                nc.sync.dma_start(wo_sb[:], wod)

                with tc.tile_pool(name="psp", bufs=2, space="PSUM") as psp, \
                     tc.tile_pool(name="psav", bufs=1, space="PSUM") as psav, \
                     tc.tile_pool(name="psrep", bufs=1, space="PSUM") as psrep:
                  blocks = ([(p, Q) for p in range(8) for Q in range(2)]
                            if "B" in phases else [])
                  psbs, av2bs, recSBs = {}, {}, {}

                  def b_qk(p, Q):
                      # scores -> exp -> 0/1 window masks (DVE+Pool split)
                      kv = p // 2
                      psb = sbB.tile([128, 3072], F32R, tag="psb",
                                     name=f"psb{p}{Q}")
                      psbs[(p, Q)] = psb
                      for h2 in range(2):
                          b = 64 * h2
                          pp = psp.tile([128, 1536], F32, tag="pp",
                                        name=f"pp{p}{Q}{h2}")
                          for nu in range(6):
                              kap = 2 * Q + nu
                              nc.tensor.matmul(
                                  pp[:, nu * 256:(nu + 1) * 256],
                                  kTd[b:b + 64,
                                      kv * 1024 + kap * 128:kv * 1024 + (kap + 1) * 128],
                                  qT_sb[p][b:b + 64, Q * 256:(Q + 1) * 256])
                          nc.scalar.activation(
                              psb[:, h2 * 1536:(h2 + 1) * 1536],
                              pp[:], AF.Exp, scale=SCALE)
                          nc.vector.tensor_mul(
                              psb[:, h2 * 1536:h2 * 1536 + 512],
                              psb[:, h2 * 1536:h2 * 1536 + 512],
                              mask01[:, 0:512])
                          nc.gpsimd.tensor_mul(
                              psb[:, h2 * 1536 + 1024:h2 * 1536 + 1536],
                              psb[:, h2 * 1536 + 1024:h2 * 1536 + 1536],
                              mask01[:, 512:1024])

                  def b_av(p, Q):
                      kv = p // 2
                      psb = psbs.pop((p, Q))
                      if Q == 0:
                          av2bs[p] = sbB.tile([65, 1024], F32R, tag="av2b",
                                              name=f"av2b{p}")
                      avp = psav.tile([65, 512], F32, tag="avp", name=f"avp{p}{Q}")
                      for h2 in range(2):
                          for nu in range(6):
                              stk = 2 * Q + nu
                              nc.tensor.matmul(
                                  avp[:, h2 * 256:(h2 + 1) * 256],
                                  v_sb[stk][:, kv * 65:(kv + 1) * 65],
                                  psb[:, h2 * 1536 + nu * 256:h2 * 1536 + (nu + 1) * 256],
                                  start=(nu == 0), stop=(nu == 5))
                      # denominator (row 0 of avp) + sink -> sbuf
                      den = sbB.tile([1, 512], F32R, tag="den", name=f"den{p}{Q}")
                      nc.vector.tensor_add(
                          den[:], avp[0:1, :],
                          sinkr[0:1, (2 * p + Q) * 512:(2 * p + Q + 1) * 512])
                      # drain unnormalized av (row 0 = den, copied but unused
                      # downstream; engine PSUM reads must start partition-aligned)
                      nc.vector.tensor_copy(av2bs[p][0:65, Q * 512:(Q + 1) * 512],
                                            avp[0:65, :])
                      # broadcast den over all partitions, wide reciprocal,
                      # then normalize av2b in place (free layout (h2, q))
                      denB = psrep.tile([128, 512], F32, tag="denB",
                                        name=f"denB{p}{Q}")
                      nc.tensor.matmul(denB[:], onesrow[:], den[:])
                      recSB = sbB.tile([65, 512], F32, tag="recSB",
                                       name=f"recSB{p}{Q}")
                      nc.vector.reciprocal(recSB[:], denB[0:65, :])
                      nc.vector.tensor_mul(
                          av2bs[p][0:65, Q * 512:(Q + 1) * 512],
                          av2bs[p][0:65, Q * 512:(Q + 1) * 512],
                          recSB[:])
                      if Q == 1:
                          # relocate into head-pair layout [128=(h2,d), 512 q]
                          av2b = av2bs.pop(p)
                          avv = av2b[1:65, :].rearrange("p (q h c) -> p q h c",
                                                        q=2, h=2)
                          nc.sync.dma_start(
                              aoT_sb[p][0:64, :].rearrange("p (q c) -> p q c", q=2),
                              avv[:, :, 0, :])
                          nc.sync.dma_start(
                              aoT_sb[p][64:128, :].rearrange("p (q c) -> p q c", q=2),
                              avv[:, :, 1, :])

                  # software pipeline: scores one block ahead of AV
                  for i, blk in enumerate(blocks):
                      b_qk(*blk)
                      if i > 0:
                          b_av(*blocks[i - 1])
                  if blocks:
                      b_av(*blocks[-1])

                with tc.tile_pool(name="psC", bufs=2, space="PSUM") as psC:
                  for sblk in (range(4) if "C" in phases else []):
                    op = psC.tile([128, 1024], F32, tag="op", name=f"op{sblk}")
                    for nh in range(2):
                        for kb in range(8):
                            nc.tensor.matmul(
                                op[:, nh * 512:(nh + 1) * 512],
                                aoT_sb[kb][:, sblk * 128:(sblk + 1) * 128],
                                wo_sb[:, kb * 1024 + nh * 512:kb * 1024 + (nh + 1) * 512],
                                start=(kb == 0), stop=(kb == 7))
                    osb = sbB.tile([128, 1024], F32, tag="osb", name=f"osb{sblk}")
                    nc.scalar.copy(osb[:], op[:])
                    nc.sync.dma_start(
                        outd[sblk * 128:(sblk + 1) * 128, :], osb[:])

    nc.compile()
    return nc


def _shuffle(a, k):
    """[k*128, F] -> [128, k*F] with cols kb*F + c."""
    p, f = 128, a.shape[1]
    return np.ascontiguousarray(
        a.reshape(k, p, f).transpose(1, 0, 2).reshape(p, k * f))


def _prep_inputs(x, cos, sin, wq, wk, wv, wo, q_norm_w, k_norm_w, sinks):
    """Build the 8 per-core input maps (bf16 staged, SBUF-layout shuffled)."""
    x = np.asarray(x, np.float32).reshape(S, HID)
    cos = np.asarray(cos, np.float32)
    sin = np.asarray(sin, np.float32)
    wq = np.asarray(wq, np.float32)
    wk = np.asarray(wk, np.float32)
    wv = np.asarray(wv, np.float32)
    wo = np.asarray(wo, np.float32)
    qw = np.asarray(q_norm_w, np.float32)
    kw = np.asarray(k_norm_w, np.float32)
    sinks = np.asarray(sinks, np.float32)

    wq_s = _shuffle(np.ascontiguousarray(wq.T), 8)                  # [128, 8192]
    wkv_s = _shuffle(np.ascontiguousarray(
        np.concatenate([wk, wv], 0).T), 8)                          # [128, 4096]
    wo_s = _shuffle(np.ascontiguousarray(wo.T), 8)                  # [128, 8192]
    ident = np.eye(128, dtype=np.float32)

    # rope coefficient tables with norm weight folded in
    sgn = np.concatenate([-np.ones(32, np.float32), np.ones(32, np.float32)])
    wrot_q = np.concatenate([qw[32:], qw[:32]])
    wrot_k = np.concatenate([kw[32:], kw[:32]])
    cw_q = cos * qw[None, :]
    sw_q = sin * (sgn * wrot_q)[None, :]
    cw_k = cos * kw[None, :]
    sw_k = sin * (sgn * wrot_k)[None, :]

    # 0/1 multiplicative window masks for partial strips, order nu=0,1,4,5
    r = np.arange(128)[:, None]
    c = np.arange(256)[None, :]
    mstack = []
    for nu in (0, 1, 4, 5):
        ij = c - r + 512 - 128 * nu
        allowed = (ij >= 0) & (ij < WINDOW)
        mstack.append(allowed.astype(np.float32))
    mask01 = np.concatenate(mstack, 1).astype(BF)                   # [128, 1024]

    xT = np.ascontiguousarray(x.T)                                  # [HID, S]
    esink = np.exp(sinks.astype(np.float64)).astype(np.float32)
    sinkr = np.zeros((1, 8192), np.float32)
    for p in range(8):
        for Q in range(2):
            for h2 in range(2):
                base = (2 * p + Q) * 512 + h2 * 256
                sinkr[0, base:base + 256] = esink[2 * p + h2]

    in_maps = []
    for core in range(NCORE):
        start = SLOC * core
        lo = start - WINDOW
        xt_loc = np.zeros((HID, SKV), np.float32)
        srclo = max(0, lo)
        xt_loc[:, srclo - lo:] = xT[:, srclo:start + SLOC]
        idx_k = np.clip(np.arange(lo, start + SLOC), 0, S - 1)
        # per-strip q rope table: [128, (st, [cq|sq])]
        cq = cw_q[start:start + SLOC].astype(BF)
        sq = sw_q[start:start + SLOC].astype(BF)
        cqsq = np.concatenate(
            [np.concatenate([cq[i * 128:(i + 1) * 128], sq[i * 128:(i + 1) * 128]], 1)
             for i in range(4)], 1)                                  # [128, 512]
        ck = cw_k[idx_k].astype(BF)
        sk = sw_k[idx_k].astype(BF)
        cksk = np.concatenate(
            [np.concatenate([ck[i * 128:(i + 1) * 128], sk[i * 128:(i + 1) * 128]], 1)
             for i in range(8)], 1)                                  # [128, 1024]
        vcol = np.ones((128, 8), BF)
        if core == 0:
            vcol[:, 0:4] = 0
        in_maps.append(dict(
            xT=_shuffle(xt_loc, 8),
            wq=wq_s, wkv=wkv_s, wo=wo_s,
            cqsq=np.ascontiguousarray(cqsq),
            cksk=np.ascontiguousarray(cksk),
            mask01=mask01, ident=ident, sinkr=sinkr, vcol=vcol,
        ))
    return in_maps


def kernel(x, cos, sin, wq, wk, wv, wo, q_norm_w, k_norm_w, sinks, **kw):
    if "nc" not in _cache:
        _cache["nc"] = _build()
    nc = _cache["nc"]
    in_maps = _prep_inputs(x, cos, sin, wq, wk, wv, wo, q_norm_w, k_norm_w, sinks)
    res = run_bass_kernel_spmd(nc, in_maps, core_ids=list(range(NCORE)), **kw)
    out = np.empty((S, HID), np.float32)
    for core in range(NCORE):
        out[core * SLOC:(core + 1) * SLOC] = res.results[core]["out"]
    if kw:
        _cache["last_results"] = res
    return out.reshape(B, S, HID)


# revision 4
# speedup vs baseline: 1.3508x; 1.3508x over previous
"""Sliding-window GQA attention (B=1, S=4096, HID=1024, H=16, KV=4, D=64, W=512)
sharded across 8 trn2 NeuronCores by sequence (512 query rows/core + 512-row
k/v halo recomputed locally).

v2: bf16 compute (DVE 2x, half the HBM bytes), merged DMAs (~34 vs 167),
window mask as post-exp 0/1 multiplies (DVE/GpSimd) instead of PE mask-add
matmuls, sink folded into the denominator (no per-block sink DMAs), rope
pair-sum folded before the PE transposes (half the transpose matmuls),
denominator broadcast via PE then one wide reciprocal, software-pipelined
attention blocks (scores run ahead of AV), weights hoisted+preloaded, and
A1/A2 sharing PSUM pools to avoid phase-boundary WAR stalls.

Self-contained: takes full inputs, shards on host, runs one SPMD Bass kernel
on cores 0-7, reassembles the full output.
"""
import sys
sys.path.insert(0, '/opt/trn_rl_repo')
import numpy as np
import ml_dtypes

import concourse.bass as bass
import concourse.bacc as bacc
import concourse.mybir as mybir
import concourse.hw_specs as _hw_specs

# Route every Ln/Exp activation to the single set that contains both
# ("natural_log_exp_and_others"), so the scheduler's interleaving of Ln and
# Exp ops never forces an ACT table reload (~2.7us each).
_orig_get_act_tables = _hw_specs.get_activation_tables


def _merged_act_tables(arch):
    t = dict(_orig_get_act_tables(arch))
    strip = {mybir.ActivationFunctionType.Ln, mybir.ActivationFunctionType.Exp,
             mybir.ActivationFunctionType.Square}
    for name, fns in t.items():
        if name != "natural_log_exp_and_others":
            t[name] = fns - strip
    return t


bacc.get_activation_tables = _merged_act_tables
import concourse.tile as tile
from concourse.bass_utils import run_bass_kernel_spmd

F32 = mybir.dt.float32
F32R = mybir.dt.float32r
BF16 = mybir.dt.bfloat16
AF = mybir.ActivationFunctionType
OP = mybir.AluOpType
BF = ml_dtypes.bfloat16

B, S, HID = 1, 4096, 1024
H, KV, D = 16, 4, 64
WINDOW = 512
EPS = 1e-5
SCALE = 0.125  # 1/sqrt(D)
NCORE = 8
SLOC = 512    # query rows per core
SKV = 1024    # k/v rows per core (halo + own)

_cache = {}


def _build(phases="ABC"):
    nc = bacc.Bacc("TRN2", target_bir_lowering=False, debug=False, num_devices=NCORE)

    xTd = nc.dram_tensor("xT", [128, 8 * SKV], F32R, kind="ExternalInput").ap()
    wqd = nc.dram_tensor("wq", [128, 8192], F32R, kind="ExternalInput").ap()
    wkvd = nc.dram_tensor("wkv", [128, 4096], F32R, kind="ExternalInput").ap()
    wod = nc.dram_tensor("wo", [128, 8192], F32R, kind="ExternalInput").ap()
    cqsqd = nc.dram_tensor("cqsq", [128, 512], BF16, kind="ExternalInput").ap()
    ckskd = nc.dram_tensor("cksk", [128, 1024], BF16, kind="ExternalInput").ap()
    maskd = nc.dram_tensor("mask01", [128, 1024], BF16, kind="ExternalInput").ap()
    identd = nc.dram_tensor("ident", [128, 128], F32R, kind="ExternalInput").ap()
    sinkrd = nc.dram_tensor("sinkr", [1, 8192], F32, kind="ExternalInput").ap()
    vcold = nc.dram_tensor("vcol", [128, 8], BF16, kind="ExternalInput").ap()
    outd = nc.dram_tensor("out", [SLOC, HID], F32, kind="ExternalOutput").ap()

    with tile.TileContext(nc) as tc:
        with tc.tile_pool(name="const", bufs=1) as cpool, \
             tc.tile_pool(name="persist", bufs=1) as pers:

            # ---------------- constants ----------------
            identb = cpool.tile([128, 128], F32R, tag="ident")
            nc.sync.dma_start(identb[:], identd)
            onesrow = cpool.tile([1, 128], F32R, tag="onesrow")
            nc.vector.memset(onesrow[:].bitcast(F32), 1.0)
            epsc = cpool.tile([128, 1], F32, tag="epsc")
            nc.vector.memset(epsc[:], EPS)
            mask01 = cpool.tile([128, 1024], BF16, tag="mask01")
            nc.sync.dma_start(mask01[:], maskd)
            sinkr = cpool.tile([1, 8192], F32, tag="sinkr")
            nc.sync.dma_start(sinkr[:], sinkrd)
            vcol = cpool.tile([128, 8], BF16, tag="vcol")
            nc.sync.dma_start(vcol[:], vcold)

            # persistent products of phase A
            # v strips: cols per kv head = [ones, d0..d63] (65 wide)
            v_sb = [pers.tile([128, 260], F32R, tag=f"v{st}", name=f"v{st}")
                    for st in range(8)]
            qT_sb = [pers.tile([128, 512], F32R, tag=f"qT{p}", name=f"qT{p}")
                     for p in range(8)]
            # kT: one tile, cols = kv*1024 + t; rows 64:128 duplicate 0:64
            kTd = pers.tile([128, 4096], F32R, tag="kTd", name="kTd")
            aoT_sb = [pers.tile([128, 512], F32R, tag=f"aoT{p}", name=f"aoT{p}")
                      for p in range(8)]

            with tc.tile_pool(name="xpool", bufs=1) as xpool:
                xT_sb = xpool.tile([128, 8 * SKV], F32R, tag="xT", name="xT")
                xv_d = xTd.rearrange("p (k t) -> p k t", k=8)
                xv_s = xT_sb[:].rearrange("p (k t) -> p k t", k=8)

                # ======== phase A1: q projection + norm + rope + transpose ====
                with tc.tile_pool(name="wqp", bufs=1) as wqp, \
                     tc.tile_pool(name="tAq", bufs=3) as tA, \
                     tc.tile_pool(name="psmmq", bufs=4, space="PSUM") as psmm, \
                     tc.tile_pool(name="pstq", bufs=1, space="PSUM") as pst:
                    wq_sb = wqp.tile([128, 8192], F32R, tag="wq", name="wq")
                    wv_d = wqd.rearrange("p (k c) -> p k c", k=8)
                    wv_s = wq_sb[:].rearrange("p (k c) -> p k c", k=8)
                    # load order: wq hf=0 half, own x, wq hf=1 half, halo x
                    nc.sync.dma_start(wv_s[:, :, 0:512], wv_d[:, :, 0:512])
                    nc.sync.dma_start(xv_s[:, :, 512:1024], xv_d[:, :, 512:1024])
                    nc.sync.dma_start(wv_s[:, :, 512:1024], wv_d[:, :, 512:1024])
                    nc.sync.dma_start(xv_s[:, :, 0:512], xv_d[:, :, 0:512])
                    cqsq = wqp.tile([128, 512], BF16, tag="cqsq", name="cqsq")
                    nc.sync.dma_start(cqsq[:], cqsqd)

                    def q_stage1(st, hf):
                        qp = psmm.tile([128, 512], F32, tag="qp", name=f"qp{st}{hf}")
                        for kb in range(8):
                            nc.tensor.matmul(
                                qp[:],
                                xT_sb[:, kb * 1024 + st * 128:kb * 1024 + (st + 1) * 128],
                                wq_sb[:, kb * 1024 + hf * 512:kb * 1024 + (hf + 1) * 512],
                                start=(kb == 0), stop=(kb == 7))
                        sqq = tA.tile([128, 512], F32, tag="sqq", name=f"sqq{st}{hf}")
                        nc.scalar.activation(sqq[:], qp[:], AF.Square)
                        msq = tA.tile([128, 8], F32, tag="msq", name=f"msq{st}{hf}")
                        nc.vector.tensor_reduce(
                            msq[:], sqq[:].rearrange("p (h d) -> p h d", h=8),
                            axis=mybir.AxisListType.X, op=OP.add)
                        return qp, msq

                    def q_stage2(st, hf, qp, msq, qTps):
                        lnq = tA.tile([128, 8], F32, tag="lnq", name=f"lnq{st}{hf}")
                        nc.scalar.activation(lnq[:], msq[:], AF.Ln, bias=epsc[:],
                                             scale=1.0 / D)
                        invq = tA.tile([128, 8], F32, tag="invq", name=f"invq{st}{hf}")
                        nc.scalar.activation(invq[:], lnq[:], AF.Exp, scale=-0.5)
                        qn = tA.tile([128, 512], F32R, tag="qn", name=f"qn{st}{hf}")
                        qnv = qn[:].rearrange("p (h d) -> p h d", h=8)
                        nc.vector.tensor_mul(
                            qnv, qp[:].rearrange("p (h d) -> p h d", h=8),
                            invq[:].unsqueeze(2).broadcast_to([128, 8, D]))
                        j = st - 4
                        ct = cqsq[:, j * 128:j * 128 + 64]
                        stt = cqsq[:, j * 128 + 64:j * 128 + 128]
                        q1 = tA.tile([128, 512], F32R, tag="q1", name=f"q1_{st}{hf}")
                        q1v = q1[:].rearrange("p (h d) -> p h d", h=8)
                        nc.vector.tensor_mul(
                            q1v, qnv, ct.unsqueeze(1).broadcast_to([128, 8, D]))
                        q2 = tA.tile([128, 512], F32R, tag="q2", name=f"q2_{st}{hf}")
                        nc.vector.tensor_mul(
                            q2[:].rearrange("p (h a j) -> p h a j", h=8, a=2),
                            qnv.rearrange("p h (a j) -> p h a j", a=2)[:, :, ::-1, :],
                            stt.rearrange("p (a j) -> p a j", a=2)
                            .unsqueeze(1).broadcast_to([128, 8, 2, 32]))
                        q12 = tA.tile([128, 512], F32R, tag="q12", name=f"q12_{st}{hf}")
                        nc.vector.tensor_add(q12[:], q1[:], q2[:])
                        for db4 in range(4):
                            nc.tensor.matmul(qTps[db4][:, j * 128:(j + 1) * 128],
                                             q12[:, db4 * 128:(db4 + 1) * 128],
                                             identb[:], is_transpose=True)

                    for hf in range(2):
                        qTps = [pst.tile([128, 512], F32R, tag=f"qTps{d}",
                                         name=f"qTps{hf}_{d}") for d in range(4)]
                        pend = None
                        for st in range(4, 8):
                            cur = (st, hf, *q_stage1(st, hf))
                            if pend is not None:
                                q_stage2(*pend, qTps)
                            pend = cur
                        q_stage2(*pend, qTps)
                        for db4 in range(4):
                            nc.vector.tensor_copy(qT_sb[hf * 4 + db4][:], qTps[db4][:])

                # ======== phase A2: k/v projection + norm + rope + transpose ====
                with tc.tile_pool(name="wkp", bufs=1) as wkp, \
                     tc.tile_pool(name="tAk", bufs=3) as tA, \
                     tc.tile_pool(name="psmmk", bufs=4, space="PSUM") as psmm, \
                     tc.tile_pool(name="pstk", bufs=1, space="PSUM") as pst:
                    wkv_sb = wkp.tile([128, 4096], F32R, tag="wkv", name="wkv")
                    nc.sync.dma_start(wkv_sb[:], wkvd)
                    cksk = wkp.tile([128, 1024], BF16, tag="cksk", name="cksk")
                    nc.sync.dma_start(cksk[:], ckskd)

                    def k_stage1(st):
                        kvp = psmm.tile([128, 512], F32, tag="qp", name=f"kvp{st}")
                        for kb in range(8):
                            nc.tensor.matmul(
                                kvp[:],
                                xT_sb[:, kb * 1024 + st * 128:kb * 1024 + (st + 1) * 128],
                                wkv_sb[:, kb * 512:(kb + 1) * 512],
                                start=(kb == 0), stop=(kb == 7))
                        # v: cols kv*65 gets ones, kv*65+1..65 gets v dims
                        nc.scalar.copy(
                            v_sb[st][:].rearrange("p (h d) -> p h d", d=65)[:, :, 1:65],
                            kvp[:, 256:512].rearrange("p (h d) -> p h d", d=64))
                        nc.vector.tensor_copy(
                            v_sb[st][:].rearrange("p (h d) -> p h d", d=65)[:, :, 0:1],
                            vcol[:, st:st + 1].unsqueeze(1).broadcast_to([128, KV, 1]))
                        sqk = tA.tile([128, 256], F32, tag="sqk", name=f"sqk{st}")
                        nc.scalar.activation(sqk[:], kvp[:, 0:256], AF.Square)
                        msk = tA.tile([128, KV], F32, tag="msk", name=f"msk{st}")
                        nc.vector.tensor_reduce(
                            msk[:], sqk[:].rearrange("p (h d) -> p h d", h=KV),
                            axis=mybir.AxisListType.X, op=OP.add)
                        return kvp, msk

                    def k_stage2(st, kvp, msk, kTps):
                        lnk = tA.tile([128, KV], F32, tag="lnk", name=f"lnk{st}")
                        nc.scalar.activation(lnk[:], msk[:], AF.Ln, bias=epsc[:],
                                             scale=1.0 / D)
                        invk = tA.tile([128, KV], F32, tag="invk", name=f"invk{st}")
                        nc.scalar.activation(invk[:], lnk[:], AF.Exp, scale=-0.5)
                        kn = tA.tile([128, 256], F32R, tag="kn", name=f"kn{st}")
                        knv = kn[:].rearrange("p (h d) -> p h d", h=KV)
                        nc.vector.tensor_mul(
                            knv, kvp[:, 0:256].rearrange("p (h d) -> p h d", h=KV),
                            invk[:].unsqueeze(2).broadcast_to([128, KV, D]))
                        ct = cksk[:, st * 128:st * 128 + 64]
                        stt = cksk[:, st * 128 + 64:st * 128 + 128]
                        k1 = tA.tile([128, 256], F32R, tag="k1", name=f"k1_{st}")
                        nc.vector.tensor_mul(
                            k1[:].rearrange("p (h d) -> p h d", h=KV),
                            knv, ct.unsqueeze(1).broadcast_to([128, KV, D]))
                        k2 = tA.tile([128, 256], F32R, tag="k2", name=f"k2_{st}")
                        nc.vector.tensor_mul(
                            k2[:].rearrange("p (h a j) -> p h a j", h=KV, a=2),
                            knv.rearrange("p h (a j) -> p h a j", a=2)[:, :, ::-1, :],
                            stt.rearrange("p (a j) -> p a j", a=2)
                            .unsqueeze(1).broadcast_to([128, KV, 2, 32]))
                        k12 = tA.tile([128, 256], F32R, tag="k12", name=f"k12_{st}")
                        nc.vector.tensor_add(k12[:], k1[:], k2[:])
                        j = st % 4
                        for kv in range(4):
                            nc.tensor.matmul(kTps[kv][:, j * 128:(j + 1) * 128],
                                             k12[:, kv * 64:(kv + 1) * 64],
                                             identb[:], is_transpose=True)

                    for sh in range(2):
                        kTps = [pst.tile([64, 512], F32R, tag=f"kTps{kv}",
                                         name=f"kTps{sh}_{kv}") for kv in range(4)]
                        pendk = None
                        for st4 in range(4):
                            st = sh * 4 + st4
                            curk = (st, *k_stage1(st))
                            if pendk is not None:
                                k_stage2(*pendk, kTps)
                            pendk = curk
                        k_stage2(*pendk, kTps)
                        for kv in range(4):
                            nc.vector.tensor_copy(
                                kTd[0:64, kv * 1024 + sh * 512:kv * 1024 + (sh + 1) * 512],
                                kTps[kv][:])
                        kv_lo = kTd[0:64, :].rearrange("p (k t) -> p k t", k=4)
                        kv_hi = kTd[64:128, :].rearrange("p (k t) -> p k t", k=4)
                        nc.sync.dma_start(kv_hi[:, :, sh * 512:(sh + 1) * 512],
                                          kv_lo[:, :, sh * 512:(sh + 1) * 512])

            # ======== phase B: attention;  phase C: out-projection ========
            with tc.tile_pool(name="wB", bufs=1) as wB, \
                 tc.tile_pool(name="sbB", bufs=2) as sbB:
                wo_sb = wB.tile([128, 8192], F32R, tag="wo", name="wo")
                nc.sync.dma_start(wo_sb[:], wod)

                with tc.tile_pool(name="psp", bufs=2, space="PSUM") as psp, \
                     tc.tile_pool(name="psav", bufs=1, space="PSUM") as psav, \
                     tc.tile_pool(name="psrep", bufs=1, space="PSUM") as psrep:
                  blocks = ([(p, Q) for p in range(8) for Q in range(2)]
                            if "B" in phases else [])
                  psbs, av2bs, recSBs = {}, {}, {}

                  def b_qk(p, Q):
                      # scores -> exp -> 0/1 window masks (DVE+Pool split)
                      kv = p // 2
                      psb = sbB.tile([128, 3072], F32R, tag="psb",
                                     name=f"psb{p}{Q}")
                      psbs[(p, Q)] = psb
                      for h2 in range(2):
                          b = 64 * h2
                          pp = psp.tile([128, 1536], F32, tag="pp",
                                        name=f"pp{p}{Q}{h2}")
                          for nu in range(6):
                              kap = 2 * Q + nu
                              nc.tensor.matmul(
                                  pp[:, nu * 256:(nu + 1) * 256],
                                  kTd[b:b + 64,
                                      kv * 1024 + kap * 128:kv * 1024 + (kap + 1) * 128],
                                  qT_sb[p][b:b + 64, Q * 256:(Q + 1) * 256])
                          nc.scalar.activation(
                              psb[:, h2 * 1536:(h2 + 1) * 1536],
                              pp[:], AF.Exp, scale=SCALE)
                          nc.vector.tensor_mul(
                              psb[:, h2 * 1536:h2 * 1536 + 512],
                              psb[:, h2 * 1536:h2 * 1536 + 512],
                              mask01[:, 0:512])
                          nc.gpsimd.tensor_mul(
                              psb[:, h2 * 1536 + 1024:h2 * 1536 + 1536],
                              psb[:, h2 * 1536 + 1024:h2 * 1536 + 1536],
                              mask01[:, 512:1024])

                  def b_av(p, Q):
                      kv = p // 2
                      psb = psbs.pop((p, Q))
                      if Q == 0:
                          av2bs[p] = sbB.tile([65, 1024], F32R, tag="av2b",
                                              name=f"av2b{p}")
                      avp = psav.tile([65, 512], F32, tag="avp", name=f"avp{p}{Q}")
                      for h2 in range(2):
                          for nu in range(6):
                              stk = 2 * Q + nu
                              nc.tensor.matmul(
                                  avp[:, h2 * 256:(h2 + 1) * 256],
                                  v_sb[stk][:, kv * 65:(kv + 1) * 65],
                                  psb[:, h2 * 1536 + nu * 256:h2 * 1536 + (nu + 1) * 256],
                                  start=(nu == 0), stop=(nu == 5))
                      # denominator (row 0 of avp) + sink -> sbuf
                      den = sbB.tile([1, 512], F32R, tag="den", name=f"den{p}{Q}")
                      nc.vector.tensor_add(
                          den[:], avp[0:1, :],
                          sinkr[0:1, (2 * p + Q) * 512:(2 * p + Q + 1) * 512])
                      # drain unnormalized av (row 0 = den, copied but unused
                      # downstream; engine PSUM reads must start partition-aligned)
                      nc.vector.tensor_copy(av2bs[p][0:65, Q * 512:(Q + 1) * 512],
                                            avp[0:65, :])
                      # broadcast den over all partitions, wide reciprocal,
                      # then normalize av2b in place (free layout (h2, q))
                      denB = psrep.tile([128, 512], F32, tag="denB",
                                        name=f"denB{p}{Q}")
                      nc.tensor.matmul(denB[:], onesrow[:], den[:])
                      recSB = sbB.tile([65, 512], F32, tag="recSB",
                                       name=f"recSB{p}{Q}")
                      nc.vector.reciprocal_approx_fast(recSB[:], denB[0:65, :])
                      nc.vector.tensor_mul(
                          av2bs[p][0:65, Q * 512:(Q + 1) * 512],
                          av2bs[p][0:65, Q * 512:(Q + 1) * 512],
                          recSB[:])
                      if Q == 1:
                          # relocate into head-pair layout [128=(h2,d), 512 q]
                          av2b = av2bs.pop(p)
                          avv = av2b[1:65, :].rearrange("p (q h c) -> p q h c",
                                                        q=2, h=2)
                          nc.sync.dma_start(
                              aoT_sb[p][0:64, :].rearrange("p (q c) -> p q c", q=2),
                              avv[:, :, 0, :])
                          nc.sync.dma_start(
                              aoT_sb[p][64:128, :].rearrange("p (q c) -> p q c", q=2),
                              avv[:, :, 1, :])

                  # software pipeline: scores one block ahead of AV
                  for i, blk in enumerate(blocks):
                      b_qk(*blk)
                      if i > 0:
                          b_av(*blocks[i - 1])
                  if blocks:
                      b_av(*blocks[-1])

                with tc.tile_pool(name="psC", bufs=2, space="PSUM") as psC:
                  for sblk in (range(4) if "C" in phases else []):
                    op = psC.tile([128, 1024], F32, tag="op", name=f"op{sblk}")
                    for nh in range(2):
                        for kb in range(8):
                            nc.tensor.matmul(
                                op[:, nh * 512:(nh + 1) * 512],
                                aoT_sb[kb][:, sblk * 128:(sblk + 1) * 128],
                                wo_sb[:, kb * 1024 + nh * 512:kb * 1024 + (nh + 1) * 512],
                                start=(kb == 0), stop=(kb == 7))
                    osb = sbB.tile([128, 1024], F32, tag="osb", name=f"osb{sblk}")
                    nc.scalar.copy(osb[:], op[:])
                    nc.sync.dma_start(
                        outd[sblk * 128:(sblk + 1) * 128, :], osb[:])

    nc.compile()
    return nc


def _shuffle(a, k):
    """[k*128, F] -> [128, k*F] with cols kb*F + c."""
    p, f = 128, a.shape[1]
    return np.ascontiguousarray(
        a.reshape(k, p, f).transpose(1, 0, 2).reshape(p, k * f))


def _prep_inputs(x, cos, sin, wq, wk, wv, wo, q_norm_w, k_norm_w, sinks):
    """Build the 8 per-core input maps (bf16 staged, SBUF-layout shuffled)."""
    x = np.asarray(x, np.float32).reshape(S, HID)
    cos = np.asarray(cos, np.float32)
    sin = np.asarray(sin, np.float32)
    wq = np.asarray(wq, np.float32)
    wk = np.asarray(wk, np.float32)
    wv = np.asarray(wv, np.float32)
    wo = np.asarray(wo, np.float32)
    qw = np.asarray(q_norm_w, np.float32)
    kw = np.asarray(k_norm_w, np.float32)
    sinks = np.asarray(sinks, np.float32)

    wq_s = _shuffle(np.ascontiguousarray(wq.T), 8)                  # [128, 8192]
    wkv_s = _shuffle(np.ascontiguousarray(
        np.concatenate([wk, wv], 0).T), 8)                          # [128, 4096]
    wo_s = _shuffle(np.ascontiguousarray(wo.T), 8)                  # [128, 8192]
    ident = np.eye(128, dtype=np.float32)

    # rope coefficient tables with norm weight folded in
    sgn = np.concatenate([-np.ones(32, np.float32), np.ones(32, np.float32)])
    wrot_q = np.concatenate([qw[32:], qw[:32]])
    wrot_k = np.concatenate([kw[32:], kw[:32]])
    cw_q = cos * qw[None, :]
    sw_q = sin * (sgn * wrot_q)[None, :]
    cw_k = cos * kw[None, :]
    sw_k = sin * (sgn * wrot_k)[None, :]

    # 0/1 multiplicative window masks for partial strips, order nu=0,1,4,5
    r = np.arange(128)[:, None]
    c = np.arange(256)[None, :]
    mstack = []
    for nu in (0, 1, 4, 5):
        ij = c - r + 512 - 128 * nu
        allowed = (ij >= 0) & (ij < WINDOW)
        mstack.append(allowed.astype(np.float32))
    mask01 = np.concatenate(mstack, 1).astype(BF)                   # [128, 1024]

    xT = np.ascontiguousarray(x.T)                                  # [HID, S]
    esink = np.exp(sinks.astype(np.float64)).astype(np.float32)
    sinkr = np.zeros((1, 8192), np.float32)
    for p in range(8):
        for Q in range(2):
            for h2 in range(2):
                base = (2 * p + Q) * 512 + h2 * 256
                sinkr[0, base:base + 256] = esink[2 * p + h2]

    in_maps = []
    for core in range(NCORE):
        start = SLOC * core
        lo = start - WINDOW
        xt_loc = np.zeros((HID, SKV), np.float32)
        srclo = max(0, lo)
        xt_loc[:, srclo - lo:] = xT[:, srclo:start + SLOC]
        idx_k = np.clip(np.arange(lo, start + SLOC), 0, S - 1)
        # per-strip q rope table: [128, (st, [cq|sq])]
        cq = cw_q[start:start + SLOC].astype(BF)
        sq = sw_q[start:start + SLOC].astype(BF)
        cqsq = np.concatenate(
            [np.concatenate([cq[i * 128:(i + 1) * 128], sq[i * 128:(i + 1) * 128]], 1)
             for i in range(4)], 1)                                  # [128, 512]
        ck = cw_k[idx_k].astype(BF)
        sk = sw_k[idx_k].astype(BF)
        cksk = np.concatenate(
            [np.concatenate([ck[i * 128:(i + 1) * 128], sk[i * 128:(i + 1) * 128]], 1)
             for i in range(8)], 1)                                  # [128, 1024]
        vcol = np.ones((128, 8), BF)
        if core == 0:
            vcol[:, 0:4] = 0
        in_maps.append(dict(
            xT=_shuffle(xt_loc, 8),
            wq=wq_s, wkv=wkv_s, wo=wo_s,
            cqsq=np.ascontiguousarray(cqsq),
            cksk=np.ascontiguousarray(cksk),
            mask01=mask01, ident=ident, sinkr=sinkr, vcol=vcol,
        ))
    return in_maps


def kernel(x, cos, sin, wq, wk, wv, wo, q_norm_w, k_norm_w, sinks, **kw):
    if "nc" not in _cache:
        _cache["nc"] = _build()
    nc = _cache["nc"]
    in_maps = _prep_inputs(x, cos, sin, wq, wk, wv, wo, q_norm_w, k_norm_w, sinks)
    res = run_bass_kernel_spmd(nc, in_maps, core_ids=list(range(NCORE)), **kw)
    out = np.empty((S, HID), np.float32)
    for core in range(NCORE):
        out[core * SLOC:(core + 1) * SLOC] = res.results[core]["out"]
    if kw:
        _cache["last_results"] = res
    return out.reshape(B, S, HID)
